# revision 13
# baseline (speedup 1.0000x reference)
"""Trainium2 Bass kernel for prefix-KV causal attention (nn_Attn_38757784879167).

Sharding: 8 cores <- (b, h) pairs (B=2 x H=4). Each core computes q/k/v
projections and S^T-layout flash attention (keys on partitions, queries on
the free dim) for one (batch, head); the host gather merges the two PV
parity partials, normalizes by the rowsums, applies the out-projection and
sums heads (the "out-projection all-reduce" step).

Per-core pipeline (vs the 198us baseline = 1.27x):
  1. exp split across engines: ScalarE runs true exp; VectorE runs a
     Schraudolph bit-trick exp (int16(trunc(psS + B)) bit-viewed as fp16,
     with scores pre-scaled by 1024*log2(e)/sqrt(hd) folded into Wq, plus a
     max(.,0) clamp). Centered magic bias keeps the sawtooth error ~+-3%,
     which softmax normalization largely cancels (rel err ~1e-2 < 2e-2).
  2. Diagonal (causal-masked) chunks always go to ScalarE; their 0/1 mask
     multiply runs on the otherwise-idle GpSimd engine, scheduled early in
     each query tile so it overlaps the prefix chunks.
  3. PV matmuls column-tiled 2x by chunk parity: accumulators in two PSUM
     banks at partition ranges 0-33 / 64-97, concurrent pairs (Dt ~ 4ns).
  4. Scores matmuls: 2x row-tiled 64-row pairs (k/q replicated at partition
     groups 0 and 2 via host-packed [w|0|w|0] projection weights) once the
     pipeline is dense; full-128-row form (both replicas -> 2x score,
     undone in the exp constants) for the early DMA-sparse query tiles,
     because low-utilization tiles let the PE HAM clock throttle to 1.2GHz.
  5. A dense dummy-matmul warmup burst during the input DMA un-throttles
     the PE clock before real work starts.
  6. Input DMA ordered + split across SP/ACT/GpSimd queues so the first
     query tile unblocks in ~6us.
"""

import math
import os

import numpy as np

B = 2
T = 4096
D = 128
H = 4
HD = 32
VE = HD + 2  # v-image stride: v (32) | ones (1) | pad (1)
PRE = 2048
CH = 128
QT = 512
GRP = 2  # chunks per scores-PSUM tile / exp batch
NPRE = PRE // CH  # 16 prefix chunks
NQT = T // QT  # 8 query tiles

SQ = 1.0 / math.sqrt(HD)
S_SCH = 1024.0 * math.log2(math.e)  # Schraudolph pre-scale (per scaled-score unit)
B0 = 0.0  # exp bias (folded into softmax normalization; 0 keeps the
# Schraudolph int16 argument positive for all scores >= -10.4 sigma)
C_CENTER = 0.0354  # Schraudolph sawtooth centering
B_ADD = 1024.0 * (15.0 + B0 * math.log2(math.e) - C_CENTER) + 0.5
ACT_SCALE = math.log(2.0) / 1024.0  # activation scale to undo S_SCH

DVE_PAT = (True, False, False, True, False, True, False, False)  # 3/8 of non-diag groups on VectorE
# (pair-mode diagonal groups also run there with the mask fused in)

_CACHE = {}


def _offsets():
    """Column offsets into the packed fp16 [128, NW] input tensor."""
    o = {}
    o["wq4"] = 0  # [128,128] q-proj weights: [wq|0|wq|0] replicas, pre-scaled
    o["wk4"] = o["wq4"] + 128  # [128,128] k-proj weights: [wk|0|wk|0]
    o["wv"] = o["wk4"] + 128  # [128,32] v-proj weights
    o["x0"] = o["wv"] + 32  # xT first query tile [128, 512]
    o["pk4"] = o["x0"] + QT  # prefix kT at partition groups 0/2, zeros at 1/3
    o["pv"] = o["pk4"] + PRE  # prefix v image [128, NPRE*VE] (keys on partitions)
    o["m"] = o["pv"] + NPRE * VE  # 0/1 masks [128, 4*512]
    o["xr"] = o["m"] + 4 * QT  # xT rest [128, T-512]
    o["nw"] = o["xr"] + (T - QT)
    return o


OFF = _offsets()


def _group_chunks(nch):
    """Chunk processing order for one query tile: the 4 diagonal (masked)
    chunks first (so the gpsimd mask multiplies overlap the rest), then the
    remaining chunks in pairs. Every pair is (even, odd) parity -> its two PV
    matmuls hit different PSUM banks AND different column-tile positions, so
    they run concurrently. Returns list of (chunks, is_diag_group)."""
    diag = list(range(nch - 4, nch))
    rest = list(range(0, nch - 4))
    rest_groups = [(rest[i : i + GRP], False) for i in range(0, len(rest), GRP)]
    # Two prefix groups first: they only need this tile's q projection, so the
    # PE has work while the kT evac / mask / gpsimd chain for the diagonal
    # groups fills. Diagonal groups next so their mask multiplies overlap the
    # remaining prefix groups.
    groups = rest_groups[:2] + [(diag[:2], True), (diag[2:], True)] + rest_groups[2:]
    return groups


def build_attn():
    from contextlib import ExitStack

    import concourse.mybir as mybir
    import concourse.tile as tile
    from concourse import bacc

    f32 = mybir.dt.float32
    fp16 = mybir.dt.float16
    i16 = mybir.dt.int16
    EXP = mybir.ActivationFunctionType.Exp
    ALU = mybir.AluOpType

    nc = bacc.Bacc("TRN2", target_bir_lowering=False, debug=False)

    pack_d = nc.dram_tensor("pack", [128, OFF["nw"]], fp16, kind="ExternalInput")
    # ctxT rows 0-33: parity-A ctx^T (rows 0-31) + rowsum (row 32);
    # rows 64-97: parity-B. Out-projection + head-sum happen on the host.
    ctxT_d = nc.dram_tensor("ctxT", [128, T], fp16, kind="ExternalOutput")

    with tile.TileContext(nc) as tc, ExitStack() as ctx:
        const = ctx.enter_context(tc.tile_pool(name="const", bufs=1))
        packed = const.tile([128, OFF["nw"]], fp16, tag="pack")
        qT4_s = const.tile([128, T], fp16, tag="qT4")
        kT4_s = const.tile([128, T], fp16, tag="kT4")  # projected keys, 4 replicas
        vS_s = const.tile([128, (T // CH) * VE], fp16, tag="vS")
        nb_s = const.tile([128, 1], f32, tag="nb")

        # ---- input DMAs: ordered so early compute unblocks fast; split across
        # engines so several hardware queues run in parallel.
        with tc.high_priority(offset=50):
            # sync (SP) queue: weights, then prefix k; scalar (ACT) queue: x
            # tile 0 in parallel so the first projections start ~4us earlier
            nc.sync.dma_start(packed[:, 0 : OFF["x0"]], pack_d[:, 0 : OFF["x0"]])
            nc.scalar.dma_start(
                packed[:, OFF["x0"] : OFF["pk4"]], pack_d[:, OFF["x0"] : OFF["pk4"]]
            )
            nc.sync.dma_start(
                packed[:, OFF["pk4"] : OFF["pv"]], pack_d[:, OFF["pk4"] : OFF["pv"]]
            )
            nc.scalar.dma_start(
                packed[:, OFF["pv"] : OFF["m"]], pack_d[:, OFF["pv"] : OFF["m"]]
            )
            # scalar (ACT) queue: x tiles 1-3
            xr_mid = OFF["xr"] + 3 * QT
            nc.scalar.dma_start(packed[:, OFF["xr"] : xr_mid], pack_d[:, OFF["xr"] : xr_mid])
            # sync queue again: x tiles 4-7 (needed latest)
            nc.sync.dma_start(packed[:, xr_mid : OFF["nw"]], pack_d[:, xr_mid : OFF["nw"]])
            # gpsimd (SWDGE) queue: masks (needed by first diag groups)
            nc.gpsimd.dma_start(packed[:, OFF["m"] : OFF["xr"]], pack_d[:, OFF["m"] : OFF["xr"]])

        nc.vector.memset(nb_s[:], B0)

        vS_3d = vS_s[:].rearrange("p (c e) -> p c e", e=VE)
        nc.gpsimd.memset(vS_3d[:, :, HD], 1.0)
        nc.gpsimd.memset(vS_3d[:, :, HD + 1], 0.0)

        # ---- PE warmup: HAM un-throttles (1.2 -> 2.4 GHz) only after a
        # fully-busy ~3.4us window, and the dependency-paced main loop never
        # provides one. Run a dense burst of dummy matmuls on a zeroed scratch
        # while the input DMA streams in; once warm, the PE stays warm (no
        # >3.4us idle gaps later).
        wq4 = packed[:, OFF["wq4"] : OFF["wq4"] + 128]
        wk4 = packed[:, OFF["wk4"] : OFF["wk4"] + 128]
        wv = packed[:, OFF["wv"] : OFF["wv"] + HD]
        mask_s = packed[:, OFF["m"] : OFF["m"] + 4 * QT]
        pvimg = packed[:, OFF["pv"] : OFF["m"]].rearrange("p (c e) -> p c e", e=VE)

        def xT_tile(j):
            if j == 0:
                return packed[:, OFF["x0"] : OFF["x0"] + QT]
            return packed[:, OFF["xr"] + (j - 1) * QT : OFF["xr"] + j * QT]

        def kT4_chunk(c, g):
            """Scores lhsT for chunk c. g=None: full 128-row contraction (both
            k replicas contribute -> 2x score, undone in the exp constants);
            keeps the PE activity monitor at the warm 2.4 GHz clock. g in
            {0,1}: 64-row slice for the 2x row-tiled pair form."""
            lo, hi = (0, 128) if g is None else (64 * g, 64 * g + 64)
            if c < NPRE:
                return packed[lo:hi, OFF["pk4"] + CH * c : OFF["pk4"] + CH * (c + 1)]
            cc = c - NPRE
            return kT4_s[lo:hi, CH * cc : CH * (cc + 1)]

        def v_chunk(c):
            if c < NPRE:
                return pvimg[:, c, :]
            return vS_3d[:, c - NPRE, :]

        psSp = ctx.enter_context(tc.tile_pool(name="psS", bufs=3, space="PSUM"))
        psCAp = ctx.enter_context(tc.tile_pool(name="psCA", bufs=1, space="PSUM"))
        psCBp = ctx.enter_context(tc.tile_pool(name="psCB", bufs=1, space="PSUM"))
        ptp = ctx.enter_context(tc.tile_pool(name="pt", bufs=4))
        ctxp = ctx.enter_context(tc.tile_pool(name="ctx", bufs=2))

        dve_ctr = [0]

        for j in range(NQT):
            nch = (PRE + QT * (j + 1)) // CH
            # Early query tiles run sparse (DMA-gated) and the 50%-utilization
            # row-tiled pairs let the PE clock flap; full-K scores hold 2.4GHz.
            fullk = j < 1

            # ---- projections for this query tile
            psP = psSp.tile([128, GRP * QT], f32, tag="s")  # q|k
            psV = psSp.tile([128, GRP * QT], f32, tag="s")  # v projections
            with tc.high_priority(offset=10):
                nc.tensor.matmul(psP[:, 0:QT], wq4, xT_tile(j))
                nc.tensor.matmul(psP[:, QT : 2 * QT], wk4, xT_tile(j))
            with tc.high_priority(offset=5):
                nc.scalar.activation(
                    qT4_s[:, QT * j : QT * (j + 1)], psP[:, 0:QT],
                    mybir.ActivationFunctionType.Copy,
                )
                nc.scalar.activation(
                    kT4_s[:, QT * j : QT * (j + 1)], psP[:, QT : 2 * QT],
                    mybir.ActivationFunctionType.Copy,
                )
            for i in range(4):
                with tc.high_priority(offset=10):
                    nc.tensor.matmul(
                        psV[:, HD * i : HD * (i + 1)],
                        xT_tile(j)[:, CH * i : CH * (i + 1)],
                        wv,
                    )
            # one strided copy evacuates all 4 v projections
            nc.vector.tensor_copy(
                vS_3d[:, 4 * j : 4 * j + 4, 0:HD],
                psV[:, 0 : 4 * HD].rearrange("p (c e) -> p c e", e=HD),
            )

            psCA = psCAp.tile([128, QT], f32, tag="ca")
            psCB = psCBp.tile([128, QT], f32, tag="cb")

            # ---- attention chunks (diag pairs first, then rest in pairs)
            groups = _group_chunks(nch)
            pv_seen = {0: 0, 1: 0}
            idx = 0  # global chunk index within this qtile -> scores row slot

            for gi, (chunks, is_diag) in enumerate(groups):
                w = len(chunks) * QT
                psS = psSp.tile([128, GRP * QT], f32, tag="s")
                pair_form = is_diag or not fullk
                # hoist each tile's first two score groups ahead of the
                # previous tile's trailing PV work to hide boundary stalls
                with tc.high_priority(offset=12 if gi < 2 else 10):
                    for i, c in enumerate(chunks):
                        g = idx % 2 if pair_form else None
                        idx += 1
                        lo, hi = (0, 128) if g is None else (64 * g, 64 * g + 64)
                        nc.tensor.matmul(
                            psS[:, QT * i : QT * (i + 1)],
                            kT4_chunk(c, g),
                            qT4_s[lo:hi, QT * j : QT * (j + 1)],
                        )
                pt = ptp.tile([128, GRP * QT], fp16, tag="pt")
                if is_diag:
                    # diag groups always use pair-form scores (psS not
                    # doubled), so the 0/1 mask fuses into the VectorE
                    # Schraudolph op: out = trunc((psS + B) * mask). B0=0
                    # keeps the int16 argument positive without a clamp.
                    use_dve = True
                else:
                    use_dve = DVE_PAT[dve_ctr[0] % len(DVE_PAT)]
                    dve_ctr[0] += 1
                if use_dve and is_diag:
                    for i, c in enumerate(chunks):
                        dd = c - (nch - 4)
                        nc.vector.scalar_tensor_tensor(
                            pt[:, QT * i : QT * (i + 1)].bitcast(i16),
                            psS[:, QT * i : QT * (i + 1)],
                            B_ADD,
                            mask_s[:, QT * dd : QT * (dd + 1)],
                            op0=ALU.add,
                            op1=ALU.mult,
                        )
                elif use_dve:
                    if fullk:
                        # halve the doubled full-K score; no clamp slot left,
                        # but scores never reach the int16 wrap threshold
                        nc.vector.tensor_scalar(
                            pt[:, 0:w].bitcast(i16),
                            psS[:, 0:w],
                            0.5,
                            B_ADD,
                            op0=ALU.mult,
                            op1=ALU.add,
                        )
                    else:
                        nc.vector.tensor_scalar(
                            pt[:, 0:w].bitcast(i16),
                            psS[:, 0:w],
                            B_ADD,
                            0.0,
                            op0=ALU.add,
                            op1=ALU.max,
                        )
                else:
                    nc.scalar.activation(
                        pt[:, 0:w], psS[:, 0:w], EXP, bias=nb_s[:],
                        scale=ACT_SCALE * (0.5 if fullk else 1.0),
                    )
                    if is_diag:
                        for i, c in enumerate(chunks):
                            dd = c - (nch - 4)
                            nc.gpsimd.tensor_tensor(
                                pt[:, QT * i : QT * (i + 1)],
                                pt[:, QT * i : QT * (i + 1)],
                                mask_s[:, QT * dd : QT * (dd + 1)],
                                op=ALU.mult,
                            )
                for i, c in enumerate(chunks):
                    par = c % 2
                    pv_seen[par] += 1
                    dst = psCA[0:34, :] if par == 0 else psCB[64:98, :]
                    nc.tensor.matmul(
                        dst,
                        v_chunk(c),
                        pt[:, QT * i : QT * (i + 1)],
                        start=(pv_seen[par] == 1),
                        stop=(pv_seen[par] == nch // 2),
                        skip_group_check=True,
                    )

            # ---- epilogue: evacuate both parity partials; host does the
            # out-projection + head sum (part of the gather/all-reduce).
            ctxAB = ctxp.tile([128, QT], fp16, tag="ctx")
            with tc.high_priority(offset=8):
                nc.vector.tensor_copy(ctxAB[0:34, :], psCA[0:34, :])
                nc.vector.tensor_copy(ctxAB[64:98, :], psCB[64:98, :])
            nc.sync.dma_start(ctxT_d[0:34, QT * j : QT * (j + 1)], ctxAB[0:34, :])
            nc.sync.dma_start(ctxT_d[64:98, QT * j : QT * (j + 1)], ctxAB[64:98, :])

    nc.compile()
    return nc


def _make_masks():
    """Multiplicative 0/1 mask for the 4 diagonal chunk offsets."""
    m = np.zeros((CH, 4 * QT), dtype=np.float16)
    p = np.arange(CH)[:, None]
    t = np.arange(QT)[None, :]
    for dd in range(4):
        m[:, QT * dd : QT * (dd + 1)] = (t >= CH * dd + p).astype(np.float16)
    return m


_MASKS = _make_masks()


def pack_inputs(x_b, pk_bh, pv_bh, wq, wk, wv):
    p = np.zeros((128, OFF["nw"]), dtype=np.float16)
    # weights: wq scaled for Schraudolph; replicas at column groups 0 and 2
    # (zeros at 1 and 3 -> the 64-row scores contraction sees zero rows 32-63)
    wq_s = (wq * (SQ * S_SCH)).astype(np.float16)
    wk_s = wk.astype(np.float16)
    for g in (0, 2):
        p[:, OFF["wq4"] + 32 * g : OFF["wq4"] + 32 * (g + 1)] = wq_s
        p[:, OFF["wk4"] + 32 * g : OFF["wk4"] + 32 * (g + 1)] = wk_s
    p[:, OFF["wv"] : OFF["wv"] + HD] = wv
    xT = x_b.T.astype(np.float16)
    p[:, OFF["x0"] : OFF["x0"] + QT] = xT[:, 0:QT]
    p[:, OFF["xr"] : OFF["nw"]] = xT[:, QT:]
    # prefix k: replicas at partition groups 0 and 2, zeros at 1 and 3
    pkT = pk_bh.T.astype(np.float16)  # [32, PRE]
    for g in (0, 2):
        p[32 * g : 32 * (g + 1), OFF["pk4"] : OFF["pk4"] + PRE] = pkT
    # prefix v image [128 keys, NPRE, HD+2]
    vimg = np.zeros((128, NPRE, VE), dtype=np.float16)
    vimg[:, :, HD] = 1.0
    vimg[:, :, 0:HD] = pv_bh.reshape(NPRE, CH, HD).transpose(1, 0, 2)
    p[:, OFF["pv"] : OFF["m"]] = vimg.reshape(128, -1)
    p[:, OFF["m"] : OFF["m"] + 4 * QT] = _MASKS
    return p


def make_in_maps(x, pk, pv, Wqkv, Wout):
    in_maps = []
    for b in range(B):
        for h in range(H):
            in_maps.append(
                {
                    "pack": pack_inputs(
                        np.asarray(x[b], dtype=np.float32),
                        np.asarray(pk[b, h], dtype=np.float32),
                        np.asarray(pv[b, h], dtype=np.float32),
                        np.asarray(Wqkv[:, h * HD : (h + 1) * HD], dtype=np.float32),
                        np.asarray(Wqkv[:, D + h * HD : D + (h + 1) * HD], dtype=np.float32),
                        np.asarray(Wqkv[:, 2 * D + h * HD : 2 * D + (h + 1) * HD], dtype=np.float32),
                    )
                }
            )
    return in_maps


def _install_ntff_shim():
    """Provide antenv.axon_hooks (absent in this image) so trace=True works."""
    import contextlib
    import ctypes
    import sys
    import types

    try:
        from antenv.axon_hooks import get_axon_ntff_profile_hook  # noqa: F401

        return True
    except ImportError:
        pass
    so_path = "/opt/axon/libaxon_pjrt.so"
    if not os.path.exists(so_path):
        return False
    lib = ctypes.CDLL(so_path)
    if not hasattr(lib, "axon_start_nrt_profile"):
        return False
    lib.axon_start_nrt_profile.argtypes = [ctypes.POINTER(ctypes.c_int64), ctypes.c_size_t]
    lib.axon_start_nrt_profile.restype = ctypes.c_int64
    lib.axon_stop_nrt_profile.argtypes = [ctypes.c_char_p]
    lib.axon_stop_nrt_profile.restype = ctypes.c_int64

    @contextlib.contextmanager
    def _hook(output_dir, device_ids):
        import jax

        jax.devices()
        if device_ids:
            ids = (ctypes.c_int64 * len(device_ids))(*device_ids)
            rc = lib.axon_start_nrt_profile(ids, len(device_ids))
        else:
            rc = lib.axon_start_nrt_profile(None, 0)
        if rc != 0:
            raise RuntimeError(f"axon_start_nrt_profile rc={rc}")
        try:
            yield
        finally:
            n = lib.axon_stop_nrt_profile(str(output_dir).encode())
            if n < 0:
                raise RuntimeError(f"axon_stop_nrt_profile rc={n}")

    mod = types.ModuleType("antenv.axon_hooks")
    mod.get_axon_ntff_profile_hook = lambda: _hook
    mod.set_axon_ntff_profile_hook = lambda h: None
    sys.modules["antenv.axon_hooks"] = mod
    return True


def kernel(x, pk, pv, Wqkv, Wout):
    from concourse.bass_utils import run_bass_kernel_spmd

    if "nc" not in _CACHE:
        _CACHE["nc"] = build_attn()
    nc = _CACHE["nc"]
    in_maps = make_in_maps(x, pk, pv, Wqkv, Wout)
    trace = bool(int(os.environ.get("ATTN_TRACE", "0")))
    if trace:
        trace = _install_ntff_shim()
    res = run_bass_kernel_spmd(nc, in_maps, core_ids=list(range(B * H)), trace=trace)
    _CACHE["last_results"] = res
    out = np.zeros((B, T, D), dtype=np.float32)
    for b in range(B):
        for h in range(H):
            r = res.results[b * H + h]
            ct = r["ctxT"].astype(np.float32)  # [128, T]
            ctx = ct[0:HD] + ct[64 : 64 + HD]  # [32, T] merged parities
            rs = ct[HD] + ct[64 + HD]  # [T]
            wout_h = np.asarray(Wout[h * HD : (h + 1) * HD, :], dtype=np.float32)
            out[b] += (wout_h.T @ (ctx / rs[None, :])).T
    return out
